# revision 21
# baseline (speedup 1.0000x reference)
import sys

for _p in ("/opt/trn_rl_repo", "/root/.axon_site/_ro/trn_rl_repo"):
    if _p not in sys.path:
        sys.path.append(_p)

import numpy as np

# Problem: B=8 batches of cross-attention-like softmax matmul, one batch per core.
#   S[e,t] = sum_d enc[e,d] * dec[t,d]
#   A = softmax(S, axis=t)
#   C[t,d] = sum_e A[e,t] * enc[e,d]
B, S, D = 8, 2048, 1024
P = 128
EB = S // P   # 16 e-blocks
TB = S // P   # 16 t-blocks (also dec cast blocks)
DC = D // P   # 8 d-chunks (contraction for scores)
TC = S // 512 # 4 t-chunks of 512 (matmul free-dim limit)

_NC_CACHE = None


def make_in_maps(enc_outputs, dec_outputs):
    ident = np.eye(P, dtype=np.float16)
    return [
        {
            "enc_outputs": np.ascontiguousarray(enc_outputs[b]),
            "dec_outputs": np.ascontiguousarray(dec_outputs[b]),
            "identc": ident,
        }
        for b in range(B)
    ]


def _build():
    import concourse.bacc as bacc
    import concourse.tile as tile
    from concourse import mybir

    F32 = mybir.dt.float32
    F16 = mybir.dt.float16

    nc = bacc.Bacc("TRN2", target_bir_lowering=False, debug=False, num_devices=B)
    enc = nc.declare_dram_parameter("enc_outputs", [S, D], F32, isOutput=False)
    dec = nc.declare_dram_parameter("dec_outputs", [S, D], F32, isOutput=False)
    identd = nc.declare_dram_parameter("identc", [P, P], F16, isOutput=False)
    out = nc.declare_dram_parameter("out", [S, D], F32, isOutput=True)

    with tile.TileContext(nc) as tc:
        with (
            tc.tile_pool(name="const", bufs=1) as const_pool,
            tc.tile_pool(name="bigT", bufs=1) as bigT_pool,
            tc.tile_pool(name="encn", bufs=1) as encn_pool,
            tc.tile_pool(name="decn", bufs=6) as decn_pool,
            tc.tile_pool(name="pmat", bufs=1) as p_pool,
            tc.tile_pool(name="stats", bufs=6) as stats_pool,
            tc.tile_pool(name="ostage", bufs=3) as out_pool,
            tc.tile_pool(name="psum_s", bufs=1, space="PSUM") as psum_s,
        ):
            # identity comes in via DRAM so gpsimd starts the cast stream
            # immediately (make_identity on gpsimd delayed the casts ~2.5us)
            ident = const_pool.tile([P, P], F16, name="ident")
            nc.sync.dma_start(out=ident[:], in_=identd[:, :])

            # d-major transposed operands, one big tile each:
            # encTbig[:, d*S + e*P + j] = enc[e*P + j, d*P + dd]  (dd = partition)
            encTbig = bigT_pool.tile([P, DC * S], F16, name="encTbig")
            decTbig = bigT_pool.tile([P, DC * S], F16, name="decTbig")
            encn = [encn_pool.tile([P, D], F16, name=f"encn{e}") for e in range(EB)]
            pmat = [p_pool.tile([P, S], F16, name=f"p{e}") for e in range(EB)]

            # ---- emission helpers ------------------------------------------
            dec_tiles = [None] * TB
            gi = [0]

            def cast_blk(which, k):
                if which == "d":
                    dtile = decn_pool.tile([P, D], F16, name="decn", tag="decn")
                    nc.gpsimd.dma_start(
                        out=dtile[:], in_=dec[k * P : (k + 1) * P, :]
                    )
                    dec_tiles[k] = dtile
                else:
                    nc.gpsimd.dma_start(
                        out=encn[k][:], in_=enc[k * P : (k + 1) * P, :]
                    )

            def t_group(which, k):
                src = dec_tiles[k] if which == "d" else encn[k]
                tgt = decTbig if which == "d" else encTbig
                tp = psum_s.tile([P, D], F16, tag="tp", bufs=2, name=f"tp{gi[0]}")
                for d in range(DC):
                    nc.tensor.transpose(
                        tp[:, d * P : (d + 1) * P], src[:, d * P : (d + 1) * P], ident
                    )
                src3 = tp[:].rearrange("p (d s) -> p d s", d=DC)
                dst3 = tgt[:].rearrange("p (d s) -> p d s", d=DC)[
                    :, :, k * P : (k + 1) * P
                ]
                nc.vector.tensor_copy(out=dst3, in_=src3)
                gi[0] += 1

            eb_state = {}  # e -> {"m_run": [tiles], "zparts": tile, "sch0": tile}

            def get_st(e):
                return eb_state.setdefault(
                    e,
                    {
                        "m_run": [None] * TC,
                        "zparts": stats_pool.tile([P, TC], F32, name=f"zp{e}", bufs=16),
                        "sch0": None,
                        "sub_done": 0,
                    },
                )

            def chunk_stats(e, j, sch):
                # online-softmax: exp against the RUNNING max so the chunk's
                # PSUM tile releases immediately (in allocation order, which
                # the round-robin slot allocator requires for pipelining);
                # earlier pmat slices get rescaled at eb end.
                st = eb_state[e]
                mj = stats_pool.tile([P, 1], F32, name="mj", bufs=48)
                nc.vector.reduce_max(out=mj, in_=sch[:], axis=mybir.AxisListType.X)
                if j == 0:
                    m_run = mj
                else:
                    m_run = stats_pool.tile([P, 1], F32, name="mrun", bufs=64)
                    nc.vector.tensor_max(out=m_run, in0=st["m_run"][j - 1], in1=mj)
                st["m_run"][j] = m_run
                negm = stats_pool.tile([P, 1], F32, name="negm", bufs=24)
                nc.vector.tensor_scalar_mul(negm, m_run, -1.0)
                nc.scalar.activation(
                    out=pmat[e][:, j * 512 : (j + 1) * 512],
                    in_=sch[:],
                    func=mybir.ActivationFunctionType.Exp,
                    bias=negm,
                    scale=1.0,
                    accum_out=st["zparts"][:, j : j + 1],
                )

            def mm_chunk(e, j):
                st = get_st(e)
                sch = psum_s.tile([P, 512], F32, tag="sps", bufs=6, name=f"s{e}_{j}")
                for d in range(DC):
                    nc.tensor.matmul(
                        sch[:],
                        lhsT=encTbig[:, d * S + e * P : d * S + (e + 1) * P],
                        rhs=decTbig[:, d * S + j * 512 : d * S + (j + 1) * 512],
                        start=(d == 0),
                        stop=(d == DC - 1),
                    )
                chunk_stats(e, j, sch)

            def mm_sub(e, q):
                # t-block-granular slice of the j=0 chunk: needs only dec
                # block q (not the whole quartet), so early e-blocks can
                # start the moment their first operands land.
                st = get_st(e)
                if st["sch0"] is None:
                    st["sch0"] = psum_s.tile(
                        [P, 512], F32, tag="sps", bufs=6, name=f"s{e}_0"
                    )
                sch = st["sch0"]
                for d in range(DC):
                    nc.tensor.matmul(
                        sch[:, q * P : (q + 1) * P],
                        lhsT=encTbig[:, d * S + e * P : d * S + (e + 1) * P],
                        rhs=decTbig[:, d * S + q * P : d * S + (q + 1) * P],
                        start=(d == 0),
                        stop=(d == DC - 1),
                    )
                st["sub_done"] += 1
                if st["sub_done"] == 4:
                    chunk_stats(e, 0, sch)

            def softmax_eb(e):
                st = eb_state[e]
                m3 = st["m_run"][TC - 1]
                for j in range(TC - 1):
                    dm = stats_pool.tile([P, 1], F32, name="dm", bufs=8)
                    nc.vector.tensor_sub(out=dm, in0=st["m_run"][j], in1=m3)
                    cj = stats_pool.tile([P, 1], F32, name="cj", bufs=8)
                    nc.scalar.activation(
                        out=cj, in_=dm, func=mybir.ActivationFunctionType.Exp,
                        bias=0.0, scale=1.0,
                    )
                    nc.vector.tensor_scalar_mul(
                        pmat[e][:, j * 512 : (j + 1) * 512],
                        pmat[e][:, j * 512 : (j + 1) * 512],
                        cj,
                    )
                    nc.vector.tensor_scalar_mul(
                        st["zparts"][:, j : j + 1], st["zparts"][:, j : j + 1], cj
                    )
                z = stats_pool.tile([P, 1], F32, name="z")
                nc.vector.reduce_sum(out=z, in_=st["zparts"][:], axis=mybir.AxisListType.X)
                zinv = stats_pool.tile([P, 1], F32, name="zinv")
                nc.vector.reciprocal(zinv, z)
                nc.vector.tensor_scalar_mul(encn[e][:], encn[e][:], zinv)

            # ---- static interleave schedule --------------------------------
            # Alternating cast order gets early e-blocks on chip fast; the
            # first dec blocks enable t-block-granular sub-chunks so the PE
            # has real work well before the first full quartet lands.
            cast_order = [
                ("e", 0), ("d", 0), ("d", 1), ("e", 1), ("d", 2), ("d", 3),
                ("e", 2), ("d", 4), ("d", 5), ("e", 3), ("d", 6), ("d", 7),
                ("e", 4), ("d", 8), ("d", 9), ("e", 5), ("d", 10), ("d", 11),
                ("e", 6), ("d", 12), ("d", 13), ("e", 7), ("d", 14), ("d", 15),
            ] + [("e", k) for k in range(8, EB)]
            SUB_ES = {0, 1, 2}  # e-blocks whose j=0 chunk is emitted per t-block

            # PE clock warm-up in the dead preamble window (HAM -> 8/8);
            # sized to run out right as the first transpose data lands.
            warm = psum_s.tile([P, 512], F32, tag="tp", bufs=2, name="warm")
            for _ in range(18):
                nc.tensor.matmul(
                    warm[:, 0:P], lhsT=ident, rhs=ident, start=True, stop=True
                )

            for w, k in cast_order:
                cast_blk(w, k)

            # Readiness-model constants (us), fit from traces.
            T0, CAST_DT = 10.0, 1.62
            TG_COST, CH_COST, SUB_COST, EVICT_LAG = 0.48, 1.76, 0.47, 1.3

            arrival = {wk: T0 + CAST_DT * (i + 1) for i, wk in enumerate(cast_order)}
            items = []  # ("SUB", e, 0, q) | ("CH", e, j, None)
            for e in range(EB):
                for j in range(TC):
                    if j == 0 and e in SUB_ES:
                        items.extend(("SUB", e, 0, q) for q in range(4))
                    else:
                        items.append(("CH", e, j, None))

            def need(it):
                kind, e, j, q = it
                if kind == "SUB":
                    return [("d", q), ("e", e)]
                return [("d", 4 * j + i) for i in range(4)] + [("e", e)]

            mmq = sorted(
                items,
                key=lambda it: (
                    max(arrival[b] for b in need(it)), it[2], it[1], it[3] or 0
                ),
            )
            tq = list(cast_order)
            t_done = {}
            pe = T0
            emitted_last_j = {}

            def chunk_ready(it):
                return all(b in t_done and t_done[b] <= pe for b in need(it))

            while tq or mmq:
                did = False
                if tq:
                    w, k = tq[0]
                    if arrival[(w, k)] <= pe or not (mmq and chunk_ready(mmq[0])):
                        pe = max(pe, arrival[(w, k)]) + TG_COST
                        t_group(w, k)
                        t_done[(w, k)] = pe + EVICT_LAG
                        tq.pop(0)
                        did = True
                if not did and mmq:
                    kind, e, j, q = mmq.pop(0)
                    if kind == "SUB":
                        pe += SUB_COST
                        mm_sub(e, q)
                    else:
                        pe += CH_COST
                        mm_chunk(e, j)
                    if j == TC - 1:
                        softmax_eb(e)

            # ---- Phase C: C[t,:] = sum_e P[e,t] * encZ[e,:] ----------------
            for t in range(TB):
                cps = [
                    psum_s.tile([P, 512], F32, tag="sps", bufs=6, name=f"c{t}_{hf}")
                    for hf in range(2)
                ]
                for e in range(EB):
                    for hf in range(2):
                        nc.tensor.matmul(
                            cps[hf][:],
                            lhsT=pmat[e][:, t * P : (t + 1) * P],
                            rhs=encn[e][:, hf * 512 : (hf + 1) * 512],
                            start=(e == 0),
                            stop=(e == EB - 1),
                        )
                o_t = out_pool.tile([P, D], F32, name="o_t")
                last = t == TB - 1
                for hf in range(2):
                    nc.any.tensor_copy(
                        out=o_t[:, hf * 512 : (hf + 1) * 512], in_=cps[hf][:]
                    )
                    # alternate the two HWDGE queues so end-of-kernel
                    # evictions drain in parallel; split the final block
                    # finer so the tail DMA is short.
                    pieces = 2 if last else 1
                    w = 512 // pieces
                    for pc in range(pieces):
                        lo = hf * 512 + pc * w
                        eng = nc.scalar if (t * 2 + hf + pc) % 2 == 0 else nc.sync
                        eng.dma_start(
                            out=out[t * P : (t + 1) * P, lo : lo + w],
                            in_=o_t[:, lo : lo + w],
                        )

    nc.compile()
    return nc


def _get_nc():
    global _NC_CACHE
    if _NC_CACHE is None:
        _NC_CACHE = _build()
    return _NC_CACHE


def kernel(enc_outputs, dec_outputs, _want_results=False, **_ignored):
    from concourse.bass_utils import run_bass_kernel_spmd

    nc = _get_nc()
    enc_outputs = np.asarray(enc_outputs, dtype=np.float32)
    dec_outputs = np.asarray(dec_outputs, dtype=np.float32)
    in_maps = make_in_maps(enc_outputs, dec_outputs)
    res = run_bass_kernel_spmd(nc, in_maps, core_ids=list(range(B)))
    out = np.stack([res.results[b]["out"] for b in range(B)], axis=0)
    if _want_results:
        return out, res
    return out
